# revision 17
# baseline (speedup 1.0000x reference)
"""Trainium2 Bass kernel for nn_AutoPruneNet (MLP policy/baseline heads + sampling).

Math (per row r of TB = T*B rows):
    h1 = relu(x @ W1.T + b1)            x: [512], h1: [400]
    h2 = relu(h1 @ W2.T + b2)           h2: [300]
    core = [h2, clip(reward,-1,1), last_action]   [302]
    pl = sigmoid(core @ Wp.T + bp)      [2]  (mu, sigma)
    baseline = core @ Wb.T + bb         [1]
    action = pl0 + pl1 * eps
    out[r] = [pl0, pl1, baseline, action]

Distribution: pure data parallel, TB rows split contiguously across 8 cores
(16384 rows each); weights replicated.

Precision: fp8(e4m3) activations + weights with DoubleRow matmuls (2 fp8
weights per PE cell -> K=256 per stream), roughly halving PE streams vs bf16.
Weights are scaled x8 on host so they sit in e4m3's normal range; the scale
compounds through the layers (psum1 = 8*y1, psum2 = 64*y2, psum_h = 64*z) and
is divided back out for free via the ACT engine's input `scale` operand.
Activations are stored as 8*h in fp8.

Layout: fc1/fc2 run feature-major [feature, row] (contraction on partitions,
zero-padded to 512 where needed — K padding costs no PE time, stream cost
depends only on N=512). The HEAD runs row-major: lhsT = core slice
[K, 128 rows] (stationary), rhs = head weights [K, 4] (moving), so the head
psum is [128 rows, (mu,sigma,base,pad)] and the whole sampling epilogue is a
handful of partition-parallel [128, 16, *] ops per 4-tile group instead of
one-partition [1,512] ops. Head biases ride as a constant-8.0 row appended to
the rwla DMA (core row 98) with 8*b head-weight entries.

Schedule: fc2 of tile t-1 is emitted after fc1 of tile t (so fc2 never waits
on the same tile's DVE relus); head phase of tile u runs at iteration u+2.
All constant inputs (weights/biases/eps) are fused into one [128, 3440]-byte
DMA (DMA preamble is packet-rate bound, ~1 packet per partition per
instruction).
"""
import sys
import types

import numpy as np
import ml_dtypes

import concourse.bacc as bacc
import concourse.bass as bass
import concourse.mybir as mybir
import concourse.tile as tile
from concourse.bass import ds, ts
from concourse.bass_utils import run_bass_kernel_spmd


def _install_ntff_hook_shim():
    """Provide the optional antenv.axon_hooks module if the image lacks it,
    so a BASS_TRACE env var in the caller can't crash run_bass_kernel_spmd.
    Registers the real NTFF profile hook when the axon .so supports it."""
    try:
        import antenv.axon_hooks  # noqa: F401
        return
    except Exception:
        pass
    try:
        import antenv
    except Exception:
        return
    mod = types.ModuleType("antenv.axon_hooks")
    state = {"hook": None}
    mod.set_axon_ntff_profile_hook = lambda h: state.__setitem__("hook", h)
    mod.get_axon_ntff_profile_hook = lambda: state["hook"]
    sys.modules["antenv.axon_hooks"] = mod
    antenv.axon_hooks = mod
    try:
        from trn_agent_boot.trn_boot import _ntff_profile_via_ctypes
        mod.set_axon_ntff_profile_hook(
            _ntff_profile_via_ctypes('/opt/axon/libaxon_pjrt.so'))
    except Exception:
        pass


_install_ntff_hook_shim()

E4 = ml_dtypes.float8_e4m3fn

N_CORES = 8
T, B, OBS = 64, 2048, 512
H1, H2 = 400, 300
TB = T * B
R = TB // N_CORES       # rows per core
NT = 512                # rows per row-tile (matmul moving dim)
OG = 4                  # row-tiles per output group
RC = NT // 128          # 128-row head chunks per tile (4)
GC = OG * RC            # head chunks per group (16)

F32 = mybir.dt.float32
F8 = mybir.dt.float8e4
AF = mybir.ActivationFunctionType
ALU = mybir.AluOpType
DR = mybir.MatmulPerfMode.DoubleRow

# fused constant-blob byte offsets (per partition); must match host_prep
OFF_W1 = 0          # [2, 2, 416] fp8
OFF_W2 = 1664       # [2, 2, 304] fp8
OFF_WH1 = 2880      # [2, 4] fp8
OFF_WH2 = 2888      # [4] fp8 (partitions 0..98 used)
OFF_B1 = 2896       # [4] f32
OFF_B2 = 2912       # [3] f32
OFF_EPS = 2928      # [rows//128] f32
WBYTES = 3440

# fc1 output (h1) chunking: {128,128,128,32}; last chunk is 16 real rows of
# h1 plus 16 zero-pad rows (weights zero) so the once-memset zero region of
# h1b's j=1 half starts at partition 32.
M1 = [(0, 128), (128, 128), (256, 128), (384, 32)]
# fc2 output (h2) chunking: {128, 128, 44}
M2 = [(0, 128), (128, 128), (256, 44)]


def build_bass(rows: int):
    """Build the per-core Bass program for `rows` rows (rows % (NT*OG) == 0)."""
    assert rows % (NT * OG) == 0
    assert rows // 128 == (WBYTES - OFF_EPS) // 4
    n_tiles = rows // NT

    nc = bacc.Bacc("TRN2", target_bir_lowering=False, debug=False)

    xt_d = nc.dram_tensor("xt", [128, n_tiles, 2, 2, NT], F8,
                          kind="ExternalInput")
    rwla_d = nc.dram_tensor("rwla", [3, rows], F8, kind="ExternalInput")
    w8_d = nc.dram_tensor("w8", [128, WBYTES], F8, kind="ExternalInput")
    out_d = nc.dram_tensor("out", [128, rows // 128, 4], F32,
                           kind="ExternalOutput")

    with tile.TileContext(nc) as tc:
        with (
            tc.tile_pool(name="w", bufs=1) as wpool,
            tc.tile_pool(name="x", bufs=4) as xpool,
            tc.tile_pool(name="h1a", bufs=4) as h1apool,
            tc.tile_pool(name="c1", bufs=6) as c1pool,
            tc.tile_pool(name="ot", bufs=2) as opool,
            tc.tile_pool(name="s", bufs=2) as spool,
            tc.tile_pool(name="ps1", bufs=3, space="PSUM") as ppool1,
            tc.tile_pool(name="ps2", bufs=3, space="PSUM") as ppool2,
            tc.tile_pool(name="ps3", bufs=2, space="PSUM") as ppool3,
        ):
            w8 = wpool.tile([128, WBYTES], F8, tag="w8")
            # w1 slice first: the first fc1 matmul only needs w1 + xt(0)
            nc.scalar.dma_start(w8[:, 0:OFF_W2], w8_d[:, 0:OFF_W2])
            nc.scalar.dma_start(w8[:, OFF_W2:], w8_d[:, OFF_W2:])
            w1_sb = w8[:, OFF_W1:OFF_W1 + 1664].rearrange(
                "p (k j m) -> p k j m", k=2, j=2, m=416)
            w2_sb = w8[:, OFF_W2:OFF_W2 + 1216].rearrange(
                "p (k j m) -> p k j m", k=2, j=2, m=304)
            wh1_sb = w8[:, OFF_WH1:OFF_WH1 + 8].rearrange(
                "p (j m) -> p j m", j=2, m=4)
            wh2_sb = w8[0:99, OFF_WH2:OFF_WH2 + 4]
            b1_sb = w8[:, OFF_B1:OFF_B1 + 16].bitcast(F32)      # [128, 4]
            b2_sb = w8[:, OFF_B2:OFF_B2 + 12].bitcast(F32)      # [128, 3]
            eps_sb = w8[:, OFF_EPS:WBYTES].bitcast(F32)         # [128, r/128]

            # Persistent rotating buffers whose zero regions are memset ONCE:
            #  h1b: j=0 -> h1 chunk2 (rewritten each tile); j=1 partitions
            #       0..31 -> h1 chunk3 (rewritten; rows 16..31 zero via zero
            #       weights); j=1 partitions 32..127 -> zero forever.
            #  c2:  99 partitions: [0:44] h2 chunk (rewritten), [44:96] zero
            #       forever, [96:99] (cr, la, const-8) DMA'd each tile.
            NB1 = 4
            NB2 = 6
            h1b_bufs, c2_bufs = [], []
            for i in range(NB1):
                hb = wpool.tile([128, 2, NT], F8, tag=f"h1b{i}")
                nc.gpsimd.memset(hb[32:64, 1, :], 0.0)
                nc.gpsimd.memset(hb[64:128, 1, :], 0.0)
                h1b_bufs.append(hb)
            for i in range(NB2):
                cb = wpool.tile([99, NT], F8, tag=f"c2{i}")
                nc.gpsimd.memset(cb[32:64, :], 0.0)
                nc.gpsimd.memset(cb[64:96, :], 0.0)
                c2_bufs.append(cb)

            utiles = {}     # tile u -> (c1, c2) for the head
            gps = {}        # group g -> (psh, obt)

            def emit_head_phase(u):
                """Emit head chunks for the rows of tile u; after the last
                phase of a group, the sampling epilogue + out DMA."""
                g, b = divmod(u, OG)
                c1, c2 = utiles.pop(u)
                if b == 0:
                    gps[g] = (ppool3.tile([128, GC, 4], F32, tag="ps3",
                                          name="psh"),
                              opool.tile([128, GC, 4], F32, tag="obt",
                                         name="obt"))
                psh, obt = gps[g]
                for q in range(RC):
                    c = RC * b + q
                    rsl = ds(q * 128, 128)
                    nc.tensor.matmul(psh[:, c, :], c2[:, rsl], wh2_sb,
                                     start=True, stop=False)
                    nc.tensor.matmul(psh[:, c, :], c1[:, 0, rsl],
                                     wh1_sb[:, 0, :], start=False, stop=False)
                    nc.tensor.matmul(psh[:, c, :], c1[:, 1, rsl],
                                     wh1_sb[:, 1, :], start=False, stop=True)
                if b == OG - 1:
                    # psum = 64*(z + b);  pl = sigmoid(z + b) etc.
                    nc.scalar.activation(obt[:, :, 0:2], psh[:, :, 0:2],
                                         AF.Sigmoid, scale=1.0 / 64.0)
                    nc.vector.tensor_scalar_mul(obt[:, :, 2], psh[:, :, 2],
                                                1.0 / 64.0)
                    se = spool.tile([128, GC], F32, tag="se")
                    nc.vector.tensor_mul(se[:], obt[:, :, 1],
                                         eps_sb[:, ds(g * GC, GC)])
                    nc.vector.tensor_add(obt[:, :, 3], obt[:, :, 0], se[:])
                    # scalar queue: keeps the out-DMA packets off the xt
                    # prefetch queue
                    nc.scalar.dma_start(out_d[:, ds(g * GC, GC), :], obt[:])
                    del gps[g]

            def emit_fc2(t, h1a, h1b, c1, c2):
                # fc2: h2T chunks {128, 128, 44}; psum = 64*y2; the m=2
                # chunk goes first so c2's assembly (relu + rwla DMA)
                # finishes earliest
                for m in (2, 0, 1):
                    m0, mw = M2[m]
                    ps2 = ppool2.tile([128, NT], F32, tag="ps2")
                    for k in range(2):
                        rhs = h1a if k == 0 else h1b
                        nc.tensor.matmul(
                            ps2[0:mw, :],
                            w2_sb[:, k, :, ds(m0, mw)],
                            rhs[:, :, :],
                            start=(k == 0),
                            stop=(k == 1),
                            perf_mode=DR,
                        )
                    # relu(64y2/8 + 8b2) on ACT -> 8*h2 in fp8
                    if m < 2:
                        nc.scalar.activation(c1[:, m, :], ps2[0:mw, :],
                                             AF.Relu,
                                             bias=b2_sb[0:mw, m:m + 1],
                                             scale=0.125)
                    else:
                        nc.scalar.activation(c2[0:44, :], ps2[0:mw, :],
                                             AF.Relu,
                                             bias=b2_sb[0:mw, m:m + 1],
                                             scale=0.125)
                utiles[t] = (c1, c2)

            fc1_out = {}    # tile t -> (h1a, h1b) for the lagged fc2
            fc2_in = {}     # tile t -> (c1, c2)

            for t in range(n_tiles + 2):
                if t < n_tiles:
                    xt_t = xpool.tile([128, 2, 2, NT], F8, tag="xt")
                    nc.sync.dma_start(xt_t[:], xt_d[:, t, :, :, :])
                    h1b = h1b_bufs[t % NB1]
                    c2 = c2_bufs[t % NB2]
                    nc.sync.dma_start(c2[96:99, :], rwla_d[:, ts(t, NT)])

                    # fc1: h1T chunks {128,128,128,32}; psum = 8*y1
                    h1a = h1apool.tile([128, 2, NT], F8, tag="h1a")
                    for c, (m0, mw) in enumerate(M1):
                        ps = ppool1.tile([128, NT], F32, tag="ps1")
                        for k in range(2):
                            nc.tensor.matmul(
                                ps[0:mw, :],
                                w1_sb[:, k, :, ds(m0, mw)],
                                xt_t[:, k, :, :],
                                start=(k == 0),
                                stop=(k == 1),
                                perf_mode=DR,
                            )
                        # relu((8y1) + 8b1) -> 8*h1 in fp8; the small
                        # chunk goes to ACT to offload the DVE
                        if c < 2:
                            dest = h1a[:, c, :]
                        elif c == 2:
                            dest = h1b[:, 0, :]
                        else:
                            dest = h1b[0:32, 1, :]
                        if c < 3:
                            nc.vector.tensor_scalar(
                                dest, ps[0:mw, :], b1_sb[0:mw, c:c + 1], 0.0,
                                ALU.add, ALU.max
                            )
                        else:
                            nc.scalar.activation(
                                dest, ps[0:mw, :], AF.Relu,
                                bias=b1_sb[0:mw, c:c + 1])
                    fc1_out[t] = (h1a, h1b)
                    fc2_in[t] = (c1pool.tile([128, 2, NT], F8, tag="c1",
                                             name="c1"), c2)

                # head of tile t-2, interleaved between fc1(t) and fc2(t-1)
                if t >= 2:
                    emit_head_phase(t - 2)

                if 1 <= t <= n_tiles:
                    h1a_p, h1b_p = fc1_out.pop(t - 1)
                    c1_p, c2_p = fc2_in.pop(t - 1)
                    emit_fc2(t - 1, h1a_p, h1b_p, c1_p, c2_p)

    nc.compile()
    return nc


def host_prep(frame, reward, last_action, eps, W1, b1, W2, b2, Wp, bp, Wb, bb,
              rows=R, n_cores=N_CORES):
    """Shard + lay out inputs for the device program. Returns in_maps."""
    frame = np.asarray(frame, np.float32).reshape(TB, OBS)
    reward = np.asarray(reward, np.float32).reshape(TB)
    la = np.asarray(last_action).reshape(TB).astype(np.float32)
    eps = np.asarray(eps, np.float32).reshape(TB)
    n_tiles = rows // NT

    W1 = np.asarray(W1, np.float32)
    W2 = np.asarray(W2, np.float32)
    b1 = np.asarray(b1, np.float32)
    b2 = np.asarray(b2, np.float32)
    Wp = np.asarray(Wp, np.float32)
    bp = np.asarray(bp, np.float32)
    Wb = np.asarray(Wb, np.float32)
    bb = np.asarray(bb, np.float32)

    # frame features f are split as f = 256k + 128j + ki
    frame_q = frame.astype(E4)          # one pass over the big tensor
    W1p = np.zeros((416, 512), np.float32)
    W1p[0:400] = 8.0 * W1
    w1_h = np.ascontiguousarray(
        W1p.T.reshape(2, 2, 128, 416).transpose(2, 0, 1, 3)).astype(E4)
    W2p = np.zeros((304, 512), np.float32)
    W2p[0:300, 0:400] = 8.0 * W2
    w2_h = np.ascontiguousarray(
        W2p.T.reshape(2, 2, 128, 304).transpose(2, 0, 1, 3)).astype(E4)

    # head weights, row-major heads: columns (mu, sigma, baseline, pad);
    # core rows: 0..255 (c1: f = 128j + ki), then c2 rows {0..43: h2
    # 256..299, 44..95: zero, 96: cr, 97: la, 98: const-8 bias row}
    Wh = np.concatenate([Wp, Wb], axis=0)           # [3, 302]
    bh = np.array([bp[0], bp[1], bb[0]], np.float32)
    wh1_h = np.zeros((128, 2, 4), np.float32)
    wh1_h[:, :, 0:3] = (8.0 * Wh[:, 0:256]).T.reshape(2, 128, 3).transpose(
        1, 0, 2)
    wh1_h = wh1_h.astype(E4)
    wh2_h = np.zeros((128, 4), np.float32)
    wh2_h[0:44, 0:3] = 8.0 * Wh[:, 256:300].T
    wh2_h[96:98, 0:3] = 8.0 * Wh[:, 300:302].T
    wh2_h[98, 0:3] = 8.0 * bh
    wh2_h = wh2_h.astype(E4)

    b1s = np.zeros(512, np.float32)
    b1s[0:400] = 8.0 * b1
    b1_h = np.ascontiguousarray(b1s.reshape(4, 128).T)
    b2s = np.zeros(384, np.float32)
    b2s[0:300] = 8.0 * b2
    b2_h = np.ascontiguousarray(b2s.reshape(3, 128).T)

    # fused constant blob (bytes), shared across cores except eps
    wbuf = np.zeros((128, WBYTES), np.uint8)
    wbuf[:, OFF_W1:OFF_W1 + 1664] = w1_h.reshape(128, 1664).view(np.uint8)
    wbuf[:, OFF_W2:OFF_W2 + 1216] = w2_h.reshape(128, 1216).view(np.uint8)
    wbuf[:, OFF_WH1:OFF_WH1 + 8] = wh1_h.reshape(128, 8).view(np.uint8)
    wbuf[:, OFF_WH2:OFF_WH2 + 4] = wh2_h.view(np.uint8)
    wbuf[:, OFF_B1:OFF_B1 + 16] = b1_h.view(np.uint8)
    wbuf[:, OFF_B2:OFF_B2 + 12] = b2_h.view(np.uint8)

    cr8 = (8.0 * np.clip(reward, -1.0, 1.0)).astype(E4)
    la8 = (8.0 * la).astype(E4)
    ones8 = np.full(TB, 8.0, np.float32).astype(E4)

    in_maps = []
    for c in range(n_cores):
        sl = slice(c * rows, (c + 1) * rows)
        xt = np.ascontiguousarray(
            frame_q[sl].T.reshape(2, 2, 128, n_tiles, NT)
            .transpose(2, 3, 0, 1, 4))
        rwla = np.stack([cr8[sl], la8[sl], ones8[sl]], axis=0)
        # eps row r lives at [r % 128, r // 128]
        eps_c = np.ascontiguousarray(eps[sl].reshape(rows // 128, 128).T)
        wb = wbuf.copy()
        wb[:, OFF_EPS:WBYTES] = eps_c.view(np.uint8)
        in_maps.append({
            "xt": xt,
            "rwla": rwla,
            "w8": wb.view(E4),
        })
    return in_maps


def assemble_out(per_core_outs):
    """[128, R//128, 4] per core (row r at [r%128, r//128]) -> [T, B, 4]."""
    outs = []
    for o in per_core_outs:
        o = np.asarray(o)
        outs.append(o.transpose(1, 0, 2).reshape(-1, B, 4))
    return np.ascontiguousarray(
        np.concatenate(outs, axis=0).astype(np.float32))


_NC_CACHE = {}


def kernel(**inputs) -> np.ndarray:
    in_maps = host_prep(**inputs)
    if R not in _NC_CACHE:
        _NC_CACHE[R] = build_bass(R)
    nc = _NC_CACHE[R]
    res = run_bass_kernel_spmd(nc, in_maps, core_ids=list(range(N_CORES)))
    return assemble_out([res.results[c]["out"] for c in range(N_CORES)])


# revision 18
# speedup vs baseline: 1.0540x; 1.0540x over previous
"""Trainium2 Bass kernel for nn_AutoPruneNet (MLP policy/baseline heads + sampling).

Math (per row r of TB = T*B rows):
    h1 = relu(x @ W1.T + b1)            x: [512], h1: [400]
    h2 = relu(h1 @ W2.T + b2)           h2: [300]
    core = [h2, clip(reward,-1,1), last_action]   [302]
    pl = sigmoid(core @ Wp.T + bp)      [2]  (mu, sigma)
    baseline = core @ Wb.T + bb         [1]
    action = pl0 + pl1 * eps
    out[r] = [pl0, pl1, baseline, action]

Distribution: pure data parallel, TB rows split contiguously across 8 cores
(16384 rows each); weights replicated.

Precision: fp8(e4m3) activations + weights with DoubleRow matmuls (2 fp8
weights per PE cell -> K=256 per stream), roughly halving PE streams vs bf16.
Weights are scaled x8 on host so they sit in e4m3's normal range; the scale
compounds through the layers (psum1 = 8*y1, psum2 = 64*y2, psum_h = 64*z) and
is divided back out for free via the ACT engine's input `scale` operand.
Activations are stored as 8*h in fp8.

Layout: fc1/fc2 run feature-major [feature, row] (contraction on partitions,
zero-padded to 512 where needed — K padding costs no PE time, stream cost
depends only on N=512). The HEAD runs row-major: lhsT = core slice
[K, 128 rows] (stationary), rhs = head weights [K, 4] (moving), so the head
psum is [128 rows, (mu,sigma,base,pad)] and the whole sampling epilogue is a
handful of partition-parallel [128, 16, *] ops per 4-tile group instead of
one-partition [1,512] ops. Head biases ride as a constant-8.0 row appended to
the rwla DMA (core row 98) with 8*b head-weight entries.

Schedule: fc2 of tile t-1 is emitted after fc1 of tile t (so fc2 never waits
on the same tile's DVE relus); head phase of tile u runs at iteration u+2.
All constant inputs (weights/biases/eps) are fused into one [128, 3440]-byte
DMA (DMA preamble is packet-rate bound, ~1 packet per partition per
instruction).
"""
import sys
import types

import numpy as np
import ml_dtypes

import concourse.bacc as bacc
import concourse.bass as bass
import concourse.mybir as mybir
import concourse.tile as tile
from concourse.bass import ds, ts
from concourse.bass_utils import run_bass_kernel_spmd


def _install_ntff_hook_shim():
    """Provide the optional antenv.axon_hooks module if the image lacks it,
    so a BASS_TRACE env var in the caller can't crash run_bass_kernel_spmd.
    Registers the real NTFF profile hook when the axon .so supports it."""
    try:
        import antenv.axon_hooks  # noqa: F401
        return
    except Exception:
        pass
    try:
        import antenv
    except Exception:
        return
    mod = types.ModuleType("antenv.axon_hooks")
    state = {"hook": None}
    mod.set_axon_ntff_profile_hook = lambda h: state.__setitem__("hook", h)
    mod.get_axon_ntff_profile_hook = lambda: state["hook"]
    sys.modules["antenv.axon_hooks"] = mod
    antenv.axon_hooks = mod
    try:
        from trn_agent_boot.trn_boot import _ntff_profile_via_ctypes
        mod.set_axon_ntff_profile_hook(
            _ntff_profile_via_ctypes('/opt/axon/libaxon_pjrt.so'))
    except Exception:
        pass


_install_ntff_hook_shim()

E4 = ml_dtypes.float8_e4m3fn

N_CORES = 8
T, B, OBS = 64, 2048, 512
H1, H2 = 400, 300
TB = T * B
R = TB // N_CORES       # rows per core
NT = 512                # rows per row-tile (matmul moving dim)
OG = 4                  # row-tiles per output group
RC = NT // 128          # 128-row head chunks per tile (4)
GC = OG * RC            # head chunks per group (16)

F32 = mybir.dt.float32
F8 = mybir.dt.float8e4
AF = mybir.ActivationFunctionType
ALU = mybir.AluOpType
DR = mybir.MatmulPerfMode.DoubleRow

# fused constant-blob byte offsets (per partition); must match host_prep
OFF_W1 = 0          # [2, 2, 416] fp8
OFF_W2 = 1664       # [2, 2, 304] fp8
OFF_WH1 = 2880      # [2, 4] fp8
OFF_WH2 = 2888      # [4] fp8 (partitions 0..98 used)
OFF_B1 = 2896       # [4] f32
OFF_B2 = 2912       # [3] f32
OFF_EPS = 2928      # [rows//128] f32
WBYTES = 3440

# fc1 output (h1) chunking: {128,128,128,32}; last chunk is 16 real rows of
# h1 plus 16 zero-pad rows (weights zero) so the once-memset zero region of
# h1b's j=1 half starts at partition 32.
M1 = [(0, 128), (128, 128), (256, 128), (384, 32)]
# fc2 output (h2) chunking: {128, 128, 44}
M2 = [(0, 128), (128, 128), (256, 44)]


def build_bass(rows: int):
    """Build the per-core Bass program for `rows` rows (rows % (NT*OG) == 0)."""
    assert rows % (NT * OG) == 0
    assert rows // 128 == (WBYTES - OFF_EPS) // 4
    n_tiles = rows // NT

    nc = bacc.Bacc("TRN2", target_bir_lowering=False, debug=False)

    xt_d = nc.dram_tensor("xt", [128, n_tiles, 2, 2, NT], F8,
                          kind="ExternalInput")
    rwla_d = nc.dram_tensor("rwla", [3, rows], F8, kind="ExternalInput")
    w8_d = nc.dram_tensor("w8", [128, WBYTES], F8, kind="ExternalInput")
    out_d = nc.dram_tensor("out", [128, rows // 128, 4], F32,
                           kind="ExternalOutput")

    with tile.TileContext(nc) as tc:
        with (
            tc.tile_pool(name="w", bufs=1) as wpool,
            tc.tile_pool(name="x", bufs=4) as xpool,
            tc.tile_pool(name="h1a", bufs=4) as h1apool,
            tc.tile_pool(name="c1", bufs=6) as c1pool,
            tc.tile_pool(name="ot", bufs=2) as opool,
            tc.tile_pool(name="s", bufs=2) as spool,
            tc.tile_pool(name="ps1", bufs=3, space="PSUM") as ppool1,
            tc.tile_pool(name="ps2", bufs=3, space="PSUM") as ppool2,
            tc.tile_pool(name="ps3", bufs=2, space="PSUM") as ppool3,
        ):
            w8 = wpool.tile([128, WBYTES], F8, tag="w8")
            # w1 slice first: the first fc1 matmul only needs w1 + xt(0)
            nc.scalar.dma_start(w8[:, 0:OFF_W2], w8_d[:, 0:OFF_W2])
            nc.scalar.dma_start(w8[:, OFF_W2:], w8_d[:, OFF_W2:])
            w1_sb = w8[:, OFF_W1:OFF_W1 + 1664].rearrange(
                "p (k j m) -> p k j m", k=2, j=2, m=416)
            w2_sb = w8[:, OFF_W2:OFF_W2 + 1216].rearrange(
                "p (k j m) -> p k j m", k=2, j=2, m=304)
            wh1_sb = w8[:, OFF_WH1:OFF_WH1 + 8].rearrange(
                "p (j m) -> p j m", j=2, m=4)
            wh2_sb = w8[0:99, OFF_WH2:OFF_WH2 + 4]
            b1_sb = w8[:, OFF_B1:OFF_B1 + 16].bitcast(F32)      # [128, 4]
            b2_sb = w8[:, OFF_B2:OFF_B2 + 12].bitcast(F32)      # [128, 3]
            eps_sb = w8[:, OFF_EPS:WBYTES].bitcast(F32)         # [128, r/128]

            # Persistent rotating buffers whose zero regions are memset ONCE:
            #  h1b: j=0 -> h1 chunk2 (rewritten each tile); j=1 partitions
            #       0..31 -> h1 chunk3 (rewritten; rows 16..31 zero via zero
            #       weights); j=1 partitions 32..127 -> zero forever.
            #  c2:  99 partitions: [0:44] h2 chunk (rewritten), [44:96] zero
            #       forever, [96:99] (cr, la, const-8) DMA'd each tile.
            NB1 = 4
            NB2 = 6
            h1b_bufs, c2_bufs = [], []
            for i in range(NB1):
                hb = wpool.tile([128, 2, NT], F8, tag=f"h1b{i}")
                nc.gpsimd.memset(hb[32:64, 1, :], 0.0)
                nc.gpsimd.memset(hb[64:128, 1, :], 0.0)
                h1b_bufs.append(hb)
            for i in range(NB2):
                cb = wpool.tile([99, NT], F8, tag=f"c2{i}")
                nc.gpsimd.memset(cb[32:64, :], 0.0)
                nc.gpsimd.memset(cb[64:96, :], 0.0)
                c2_bufs.append(cb)

            utiles = {}     # tile u -> (c1, c2) for the head
            gps = {}        # group g -> (psh, obt)

            def emit_head_phase(u):
                """Emit head chunks for the rows of tile u; after the last
                phase of a group, the sampling epilogue + out DMA."""
                g, b = divmod(u, OG)
                c1, c2 = utiles.pop(u)
                if b == 0:
                    gps[g] = (ppool3.tile([128, GC, 4], F32, tag="ps3",
                                          name="psh"),
                              opool.tile([128, GC, 4], F32, tag="obt",
                                         name="obt"))
                psh, obt = gps[g]
                for q in range(RC):
                    c = RC * b + q
                    rsl = ds(q * 128, 128)
                    nc.tensor.matmul(psh[:, c, :], c2[:, rsl], wh2_sb,
                                     start=True, stop=False)
                    nc.tensor.matmul(psh[:, c, :], c1[:, 0, rsl],
                                     wh1_sb[:, 0, :], start=False, stop=False)
                    nc.tensor.matmul(psh[:, c, :], c1[:, 1, rsl],
                                     wh1_sb[:, 1, :], start=False, stop=True)
                if b == OG - 1:
                    # psum = 64*(z + b);  pl = sigmoid(z + b) etc.
                    nc.scalar.activation(obt[:, :, 0:2], psh[:, :, 0:2],
                                         AF.Sigmoid, scale=1.0 / 64.0)
                    nc.vector.tensor_scalar_mul(obt[:, :, 2], psh[:, :, 2],
                                                1.0 / 64.0)
                    se = spool.tile([128, GC], F32, tag="se")
                    nc.vector.tensor_mul(se[:], obt[:, :, 1],
                                         eps_sb[:, ds(g * GC, GC)])
                    nc.vector.tensor_add(obt[:, :, 3], obt[:, :, 0], se[:])
                    nc.sync.dma_start(out_d[:, ds(g * GC, GC), :], obt[:])
                    del gps[g]

            def emit_fc2(t, h1a, h1b, c1, c2):
                # fc2: h2T chunks {128, 128, 44}; psum = 64*y2; the m=2
                # chunk goes first so c2's assembly (relu + rwla DMA)
                # finishes earliest
                for m in (2, 0, 1):
                    m0, mw = M2[m]
                    ps2 = ppool2.tile([128, NT], F32, tag="ps2")
                    for k in range(2):
                        rhs = h1a if k == 0 else h1b
                        nc.tensor.matmul(
                            ps2[0:mw, :],
                            w2_sb[:, k, :, ds(m0, mw)],
                            rhs[:, :, :],
                            start=(k == 0),
                            stop=(k == 1),
                            perf_mode=DR,
                        )
                    # relu(64y2/8 + 8b2) on ACT -> 8*h2 in fp8
                    if m < 2:
                        nc.scalar.activation(c1[:, m, :], ps2[0:mw, :],
                                             AF.Relu,
                                             bias=b2_sb[0:mw, m:m + 1],
                                             scale=0.125)
                    else:
                        nc.scalar.activation(c2[0:44, :], ps2[0:mw, :],
                                             AF.Relu,
                                             bias=b2_sb[0:mw, m:m + 1],
                                             scale=0.125)
                utiles[t] = (c1, c2)

            fc1_out = {}    # tile t -> (h1a, h1b) for the lagged fc2
            fc2_in = {}     # tile t -> (c1, c2)

            for t in range(n_tiles + 2):
                if t < n_tiles:
                    xt_t = xpool.tile([128, 2, 2, NT], F8, tag="xt")
                    nc.sync.dma_start(xt_t[:], xt_d[:, t, :, :, :])
                    h1b = h1b_bufs[t % NB1]
                    c2 = c2_bufs[t % NB2]
                    nc.sync.dma_start(c2[96:99, :], rwla_d[:, ts(t, NT)])

                    # fc1: h1T chunks {128,128,128,32}; psum = 8*y1
                    h1a = h1apool.tile([128, 2, NT], F8, tag="h1a")
                    for c, (m0, mw) in enumerate(M1):
                        ps = ppool1.tile([128, NT], F32, tag="ps1")
                        for k in range(2):
                            nc.tensor.matmul(
                                ps[0:mw, :],
                                w1_sb[:, k, :, ds(m0, mw)],
                                xt_t[:, k, :, :],
                                start=(k == 0),
                                stop=(k == 1),
                                perf_mode=DR,
                            )
                        # relu((8y1) + 8b1) -> 8*h1 in fp8; the small
                        # chunk goes to ACT to offload the DVE
                        if c < 2:
                            dest = h1a[:, c, :]
                        elif c == 2:
                            dest = h1b[:, 0, :]
                        else:
                            dest = h1b[0:32, 1, :]
                        if c < 3:
                            nc.vector.tensor_scalar(
                                dest, ps[0:mw, :], b1_sb[0:mw, c:c + 1], 0.0,
                                ALU.add, ALU.max
                            )
                        else:
                            nc.scalar.activation(
                                dest, ps[0:mw, :], AF.Relu,
                                bias=b1_sb[0:mw, c:c + 1])
                    fc1_out[t] = (h1a, h1b)
                    fc2_in[t] = (c1pool.tile([128, 2, NT], F8, tag="c1",
                                             name="c1"), c2)

                # head of tile t-2, interleaved between fc1(t) and fc2(t-1)
                if t >= 2:
                    emit_head_phase(t - 2)

                if 1 <= t <= n_tiles:
                    h1a_p, h1b_p = fc1_out.pop(t - 1)
                    c1_p, c2_p = fc2_in.pop(t - 1)
                    emit_fc2(t - 1, h1a_p, h1b_p, c1_p, c2_p)

    nc.compile()
    return nc


def host_prep(frame, reward, last_action, eps, W1, b1, W2, b2, Wp, bp, Wb, bb,
              rows=R, n_cores=N_CORES):
    """Shard + lay out inputs for the device program. Returns in_maps."""
    frame = np.asarray(frame, np.float32).reshape(TB, OBS)
    reward = np.asarray(reward, np.float32).reshape(TB)
    la = np.asarray(last_action).reshape(TB).astype(np.float32)
    eps = np.asarray(eps, np.float32).reshape(TB)
    n_tiles = rows // NT

    W1 = np.asarray(W1, np.float32)
    W2 = np.asarray(W2, np.float32)
    b1 = np.asarray(b1, np.float32)
    b2 = np.asarray(b2, np.float32)
    Wp = np.asarray(Wp, np.float32)
    bp = np.asarray(bp, np.float32)
    Wb = np.asarray(Wb, np.float32)
    bb = np.asarray(bb, np.float32)

    # frame features f are split as f = 256k + 128j + ki
    frame_q = frame.astype(E4)          # one pass over the big tensor
    W1p = np.zeros((416, 512), np.float32)
    W1p[0:400] = 8.0 * W1
    w1_h = np.ascontiguousarray(
        W1p.T.reshape(2, 2, 128, 416).transpose(2, 0, 1, 3)).astype(E4)
    W2p = np.zeros((304, 512), np.float32)
    W2p[0:300, 0:400] = 8.0 * W2
    w2_h = np.ascontiguousarray(
        W2p.T.reshape(2, 2, 128, 304).transpose(2, 0, 1, 3)).astype(E4)

    # head weights, row-major heads: columns (mu, sigma, baseline, pad);
    # core rows: 0..255 (c1: f = 128j + ki), then c2 rows {0..43: h2
    # 256..299, 44..95: zero, 96: cr, 97: la, 98: const-8 bias row}
    Wh = np.concatenate([Wp, Wb], axis=0)           # [3, 302]
    bh = np.array([bp[0], bp[1], bb[0]], np.float32)
    wh1_h = np.zeros((128, 2, 4), np.float32)
    wh1_h[:, :, 0:3] = (8.0 * Wh[:, 0:256]).T.reshape(2, 128, 3).transpose(
        1, 0, 2)
    wh1_h = wh1_h.astype(E4)
    wh2_h = np.zeros((128, 4), np.float32)
    wh2_h[0:44, 0:3] = 8.0 * Wh[:, 256:300].T
    wh2_h[96:98, 0:3] = 8.0 * Wh[:, 300:302].T
    wh2_h[98, 0:3] = 8.0 * bh
    wh2_h = wh2_h.astype(E4)

    b1s = np.zeros(512, np.float32)
    b1s[0:400] = 8.0 * b1
    b1_h = np.ascontiguousarray(b1s.reshape(4, 128).T)
    b2s = np.zeros(384, np.float32)
    b2s[0:300] = 8.0 * b2
    b2_h = np.ascontiguousarray(b2s.reshape(3, 128).T)

    # fused constant blob (bytes), shared across cores except eps
    wbuf = np.zeros((128, WBYTES), np.uint8)
    wbuf[:, OFF_W1:OFF_W1 + 1664] = w1_h.reshape(128, 1664).view(np.uint8)
    wbuf[:, OFF_W2:OFF_W2 + 1216] = w2_h.reshape(128, 1216).view(np.uint8)
    wbuf[:, OFF_WH1:OFF_WH1 + 8] = wh1_h.reshape(128, 8).view(np.uint8)
    wbuf[:, OFF_WH2:OFF_WH2 + 4] = wh2_h.view(np.uint8)
    wbuf[:, OFF_B1:OFF_B1 + 16] = b1_h.view(np.uint8)
    wbuf[:, OFF_B2:OFF_B2 + 12] = b2_h.view(np.uint8)

    cr8 = (8.0 * np.clip(reward, -1.0, 1.0)).astype(E4)
    la8 = (8.0 * la).astype(E4)
    ones8 = np.full(TB, 8.0, np.float32).astype(E4)

    in_maps = []
    for c in range(n_cores):
        sl = slice(c * rows, (c + 1) * rows)
        xt = np.ascontiguousarray(
            frame_q[sl].T.reshape(2, 2, 128, n_tiles, NT)
            .transpose(2, 3, 0, 1, 4))
        rwla = np.stack([cr8[sl], la8[sl], ones8[sl]], axis=0)
        # eps row r lives at [r % 128, r // 128]
        eps_c = np.ascontiguousarray(eps[sl].reshape(rows // 128, 128).T)
        wb = wbuf.copy()
        wb[:, OFF_EPS:WBYTES] = eps_c.view(np.uint8)
        in_maps.append({
            "xt": xt,
            "rwla": rwla,
            "w8": wb.view(E4),
        })
    return in_maps


def assemble_out(per_core_outs):
    """[128, R//128, 4] per core (row r at [r%128, r//128]) -> [T, B, 4]."""
    outs = []
    for o in per_core_outs:
        o = np.asarray(o)
        outs.append(o.transpose(1, 0, 2).reshape(-1, B, 4))
    return np.ascontiguousarray(
        np.concatenate(outs, axis=0).astype(np.float32))


_NC_CACHE = {}


def kernel(**inputs) -> np.ndarray:
    in_maps = host_prep(**inputs)
    if R not in _NC_CACHE:
        _NC_CACHE[R] = build_bass(R)
    nc = _NC_CACHE[R]
    res = run_bass_kernel_spmd(nc, in_maps, core_ids=list(range(N_CORES)))
    return assemble_out([res.results[c]["out"] for c in range(N_CORES)])
